# revision 54
# baseline (speedup 1.0000x reference)
"""Multi-head self-attention (RoPE + causal SDPA) Trainium2 Bass kernel.

Sharding: 8 cores = 4 batches x 2 head-groups (8 heads each).
Per core: qkv projection (tensor-parallel columns), RoPE, attention for its
8 heads, out_proj against its 512 input columns -> partial y [S, D].
Host sums the two partials per batch (pairwise reduce) and stacks batches.

QKV projections run in fp8-e4m3 DoubleRow perf mode (2 K-tiles per
instruction, 0.5 cycles/row) with a host-side hi/lo residual split:
x = 8x_hi + 8x_lo, W = 32W_hi + 32W_lo (fp8 each), computing
hi@hi (paired K-tiles) + the (hi,lo)/(lo,hi) cross terms and dropping
lo@lo. This is bf16-accuracy at ~2/3 the matmul cycles; the 256x psum
scale is folded into the rope tables and the v-store copy scale.

Layouts on device (per core):
  xT    [D, S]   bf16  hidden[b].T
  wqkT  [D, 1024] bf16 pair-interleaved cols [q-pair0 |k-pair0 |q-pair1 |...];
                      within a 128-col block: head-even feats (rope-permuted)
                      0:64, head-odd 64:128
  wvT   [D, 512] bf16  v weights (unpermuted), local-head-major
  woutT [512, D] bf16  rows match attnT feature order
  cosT2/sinT2 [128, S] bf16  rope tables in permuted layout, tiled x2 for pairs
  y     [S, D]   bf16  partial output (host upconverts and adds pairs in f32)

RoPE permuted layout: per-head feats reordered [0,2,..62, 1,3,..63] so
rotate_half becomes a 32-partition block swap, implemented as a 128x128
block-diagonal matmul (p2t) on (q * sinT2).

Attention is computed transposed: scoresT[j] = kT_j^T @ qT -> [Sk=128, Sq<=512]
so softmax's reduction axis is the matmul contraction axis; exp via ACT;
denominator via an appended ones-column in the PV lhsT ([v | 1] -> psum row 64);
PV gives attnT [64, Sq] feature-major which feeds out_proj directly.
"""

import functools
import os
import sys

import numpy as np
import ml_dtypes

try:
    import concourse.bacc as bacc
except ImportError:
    for _p in ("/opt/trn_rl_repo", "/root/.axon_site/_ro/trn_rl_repo"):
        if os.path.isdir(_p) and _p not in sys.path:
            sys.path.append(_p)
    import concourse.bacc as bacc
import concourse.mybir as mybir
import concourse.tile as tile
from concourse.bass_utils import run_bass_kernel_spmd

BF16 = mybir.dt.bfloat16
F32 = mybir.dt.float32
F8 = mybir.dt.float8e4
DR = mybir.MatmulPerfMode.DoubleRow
PSCALE = 256.0  # host scales x by 8 and w by 32; rope tables and v-copy unscale

B, S, D = 4, 1024, 1024
H, DH = 16, 64
HLOC, NPAIR = 8, 4          # heads / head-pairs per core
KT = 8                      # Sk tiles of 128
DT = 8                      # contraction (D) tiles of 128
HALF = 512                  # Sq half width (psum bank = 512 f32)
SCALE = DH ** -0.5
NEG = -1e30
MAX_POS = 4096
ROPE_BASE = 10000.0
NCORES = 8

LAST_EXEC_NS = None
LAST_TRACE = None
LAST_MODE = None

_PERM = np.concatenate([np.arange(0, DH, 2), np.arange(1, DH, 2)])  # even|odd


def _p2t_np():
    """(block-diag rotation P, transposed) P @ q' = [-q'[32:64]; q'[0:32]] per 64-block."""
    p2 = np.zeros((128, 128), np.float32)
    for b in (0, 64):
        for i in range(32):
            p2[b + i, b + 32 + i] = -1.0
            p2[b + 32 + i, b + i] = 1.0
    return np.ascontiguousarray(p2.T).astype(ml_dtypes.bfloat16)


def _mask01_np():
    """Multiplicative diag mask (bf16): masked (k>q) -> 0, allowed -> 1."""
    r = np.arange(128)
    return np.where(r[:, None] > r[None, :], 0.0, 1.0).astype(ml_dtypes.bfloat16)


def _build_program(mode):
    """mode: 'causal' | 'nomask' | 'general'"""
    nc = bacc.Bacc(None, target_bir_lowering=False)
    # fp8 hi/lo pairs: x dim1 = (lo, hi), w dim1 = (hi, lo) so DoubleRow
    # cross products pair (Whi,xlo)+(Wlo,xhi) in one instruction
    xT8 = nc.dram_tensor("xT8", [D, 2, S], F8, kind="ExternalInput")
    wqkT8 = nc.dram_tensor("wqkT8", [D, 2, 1024], F8, kind="ExternalInput")
    wvT8 = nc.dram_tensor("wvT8", [D, 2, 512], F8, kind="ExternalInput")
    woutT = nc.dram_tensor("woutT", [512, D], BF16, kind="ExternalInput")
    cosT2 = nc.dram_tensor("cosT2", [128, S], BF16, kind="ExternalInput")
    sinT2 = nc.dram_tensor("sinT2", [128, S], BF16, kind="ExternalInput")
    # y partials in bf16: halves the output DMA bytes (the drain tail is
    # DMA-serialized); the host upconverts and sums pairs in f32
    y = nc.dram_tensor("y", [S, D], BF16, kind="ExternalOutput")
    maskb = None
    if mode == "general":
        maskb = nc.dram_tensor("maskb", [S, S], BF16, kind="ExternalInput")
    debug = bool(os.environ.get("KERNEL_DEBUG"))
    dbg = {}
    if debug:
        dbg["q"] = nc.dram_tensor("dbg_q", [128, NPAIR, S], BF16, kind="ExternalOutput")
        dbg["k"] = nc.dram_tensor("dbg_k", [128, NPAIR, S], BF16, kind="ExternalOutput")
        dbg["v"] = nc.dram_tensor("dbg_v", [128, KT, HLOC, 65], BF16, kind="ExternalOutput")
        dbg["at"] = nc.dram_tensor("dbg_at", [128, NPAIR, S], BF16, kind="ExternalOutput")
        dbg["un"] = nc.dram_tensor("dbg_un", [128, NPAIR, 2, S], F32, kind="ExternalOutput")
    p2t = nc.inline_tensor(_p2t_np(), name="p2t")
    m01 = nc.inline_tensor(_mask01_np(), name="mask01")

    causal = mode == "causal"

    def active_j(h2):
        return range(4 * (h2 + 1)) if causal else range(KT)

    def col0(j, h2):
        return max(0, j * 128 - h2 * HALF) if causal else 0

    with tile.TileContext(nc) as tc:
        with tc.tile_pool(name="persist", bufs=1) as pp:
            xT_sb = pp.tile([128, 2, DT, S], F8, tag="xT")
            wqkT_sb = pp.tile([128, 2, DT, 1024], F8, tag="wqkT")
            wvT_sb = pp.tile([128, 2, DT, 512], F8, tag="wvT")
            woutT_sb = pp.tile([128, NPAIR, D], BF16, tag="woutT")
            cos_sb = pp.tile([128, S], BF16, tag="cos")
            sin_sb = pp.tile([128, S], BF16, tag="sin")
            p2t_sb = pp.tile([128, 128], BF16, tag="p2t")
            m01_sb = pp.tile([128, 128], BF16, tag="m01")
            qT_sb = pp.tile([128, NPAIR, S], BF16, tag="qT")
            kT_sb = pp.tile([128, NPAIR, S], BF16, tag="kT")
            v_sb = pp.tile([128, KT, HLOC, 65], BF16, tag="v")
            attnT_sb = pp.tile([128, NPAIR, S], BF16, tag="attnT")
            y_sb = pp.tile([128, KT, D], BF16, tag="y")
            mb_sb = None
            un_dump = None
            if debug:
                un_dump = pp.tile([128, NPAIR, 2, S], F32, tag="und")

            # input DMAs, batched + consumption-ordered: each DMA costs
            # ~625ns of serial HWDGE time + ~2.2us fixed latency, so batch
            # into ~14 transfers ordered by first use
            xT_d = xT8.ap().rearrange("(t p) l s -> p l t s", p=128)
            wqkT_d = wqkT8.ap().rearrange("(t p) l n -> p l t n", p=128)
            wvT_d = wvT8.ap().rearrange("(t p) l n -> p l t n", p=128)
            woutT_d = woutT.ap().rearrange("(p r) n -> r p n", r=128)
            # DMA APs are limited to 3 dims: one transfer per hi/lo plane,
            # hi planes first (the DoubleRow main products read only hi)
            nc.sync.dma_start(out=wqkT_sb[:, 0, :, 0:512],
                              in_=wqkT_d[:, 0, :, 0:512])
            nc.sync.dma_start(out=xT_sb[:, 1, 0:4, 0:HALF],
                              in_=xT_d[:, 1, 0:4, 0:HALF])
            nc.sync.dma_start(out=xT_sb[:, 1, 4:8, 0:HALF],
                              in_=xT_d[:, 1, 4:8, 0:HALF])
            nc.sync.dma_start(out=wqkT_sb[:, 1, :, 0:512],
                              in_=wqkT_d[:, 1, :, 0:512])
            nc.sync.dma_start(out=xT_sb[:, 0, :, 0:HALF],
                              in_=xT_d[:, 0, :, 0:HALF])
            nc.sync.dma_start(out=p2t_sb[:, :], in_=p2t.ap())
            nc.sync.dma_start(out=cos_sb[:, 0:HALF], in_=cosT2.ap()[:, 0:HALF])
            nc.sync.dma_start(out=sin_sb[:, 0:HALF], in_=sinT2.ap()[:, 0:HALF])
            nc.sync.dma_start(out=m01_sb[:, :], in_=m01.ap())
            nc.sync.dma_start(out=wvT_sb[:, 0, :, :], in_=wvT_d[:, 0, :, :])
            nc.sync.dma_start(out=wvT_sb[:, 1, :, :], in_=wvT_d[:, 1, :, :])
            nc.sync.dma_start(out=xT_sb[:, 1, :, HALF:S],
                              in_=xT_d[:, 1, :, HALF:S])
            nc.sync.dma_start(out=xT_sb[:, 0, :, HALF:S],
                              in_=xT_d[:, 0, :, HALF:S])
            nc.sync.dma_start(out=cos_sb[:, HALF:S], in_=cosT2.ap()[:, HALF:S])
            nc.sync.dma_start(out=sin_sb[:, HALF:S], in_=sinT2.ap()[:, HALF:S])
            nc.sync.dma_start(out=wqkT_sb[:, 0, :, 512:1024],
                              in_=wqkT_d[:, 0, :, 512:1024])
            nc.sync.dma_start(out=wqkT_sb[:, 1, :, 512:1024],
                              in_=wqkT_d[:, 1, :, 512:1024])
            if mode == "general":
                mb_sb = pp.tile([128, KT, S], BF16, tag="mb")
                nc.sync.dma_start(
                    out=mb_sb[:, :, :],
                    in_=maskb.ap().rearrange("(t p) s -> p t s", p=128),
                )
            nc.sync.dma_start(out=woutT_sb[:, :, :], in_=woutT_d)

            # ones column for the PV denominator rows
            nc.vector.memset(v_sb[:, :, :, 64:65], 1.0)
            warm_sb = None
            if causal:
                # p-state warmup fodder: PE ramps 0.65 -> 1.2 -> 2.4 GHz over
                # ~3us of continuous execution; burn the DMA-latency window
                # on dummy matmuls so real work runs at full speed
                warm_sb = pp.tile([128, 512], BF16, tag="warm")
                nc.vector.memset(warm_sb[:, :], 0.0)

            with tc.tile_pool(name="qkv_ps", bufs=2, space="PSUM") as qkv_ps, \
                 tc.tile_pool(name="qk_ps", bufs=2, space="PSUM") as qk_ps, \
                 tc.tile_pool(name="pv_ps", bufs=1, space="PSUM") as pv_ps, \
                 tc.tile_pool(name="rope_tmp", bufs=2 if mode == "general" else 4) as rope_tmp, \
                 tc.tile_pool(name="exp_sb", bufs=3 if mode == "general" else 8) as exp_sb, \
                 tc.tile_pool(name="mtmp", bufs=2) as mtmp, \
                 tc.tile_pool(name="norm", bufs=2 if mode == "general" else 4) as norm:

                def v_mains(st, skip=False, pool=None):
                    # alternate between the qkv slots and the (still idle)
                    # attention qk slots so four accumulators rotate
                    if pool == "qk":
                        ps2 = qk_ps.tile([128, 2, HALF], F32, tag="qk",
                                         name=f"vp{st}")
                        ps = ps2[:, 0, :]
                    elif st % 2 == 0:
                        ps = qkv_ps.tile([128, HALF], F32, tag="qkv",
                                         name=f"vp{st}")
                    else:
                        ps2 = qk_ps.tile([128, 2, HALF], F32, tag="qk",
                                         name=f"vp{st}")
                        ps = ps2[:, 0, :]
                    sl = slice(st * 128, (st + 1) * 128)
                    for dp in range(DT // 2):
                        nc.tensor.matmul(
                            ps[:, :],
                            lhsT=xT_sb[:, 1, 2 * dp:2 * dp + 2, sl],
                            rhs=wvT_sb[:, 0, 2 * dp:2 * dp + 2, :],
                            start=(dp == 0), stop=False, perf_mode=DR,
                            skip_group_check=skip)
                    return ps

                def v_crosses(st, ps, skip=False):
                    sl = slice(st * 128, (st + 1) * 128)
                    for d in range(DT):
                        nc.tensor.matmul(
                            ps[:, :],
                            lhsT=xT_sb[:, :, d, sl],
                            rhs=wvT_sb[:, :, d, :],
                            start=False, stop=(d == DT - 1), perf_mode=DR,
                            skip_group_check=skip)
                    nc.scalar.activation(
                        v_sb[:, st, :, 0:64],
                        ps.rearrange("p (h f) -> p h f", h=HLOC),
                        mybir.ActivationFunctionType.Copy, scale=1.0 / PSCALE)

                def v_proj_step(st):
                    v_crosses(st, v_mains(st))

                def qk_mains(ft, h2, skip=False):
                    pr = ft % 4
                    # wqkT is pair-interleaved: [q-pair0 |k-pair0 |q-pair1 ...]
                    qk_col = pr * 256 + (0 if ft < 4 else 128)
                    q0 = h2 * HALF
                    ps = qkv_ps.tile([128, HALF], F32, tag="qkv",
                                     name=f"qkp{ft}_{h2}")
                    qsl = slice(qk_col, qk_col + 128)
                    for dp in range(DT // 2):
                        nc.tensor.matmul(
                            ps[:, :],
                            lhsT=wqkT_sb[:, 0, 2 * dp:2 * dp + 2, qsl],
                            rhs=xT_sb[:, 1, 2 * dp:2 * dp + 2, q0:q0 + HALF],
                            start=(dp == 0), stop=False, perf_mode=DR,
                            skip_group_check=skip)
                    return ps

                def qk_crosses(ft, h2, ps, skip=False):
                    pr = ft % 4
                    qk_col = pr * 256 + (0 if ft < 4 else 128)
                    q0 = h2 * HALF
                    qsl = slice(qk_col, qk_col + 128)
                    for d in range(DT):
                        nc.tensor.matmul(
                            ps[:, :],
                            lhsT=wqkT_sb[:, :, d, qsl],
                            rhs=xT_sb[:, :, d, q0:q0 + HALF],
                            start=False, stop=(d == DT - 1), perf_mode=DR,
                            skip_group_check=skip)

                def qk_rope_finish(ft, h2, ps):
                    dest = qT_sb if ft < 4 else kT_sb
                    pr = ft % 4
                    q0 = h2 * HALF
                    u = rope_tmp.tile([128, HALF], BF16, tag="u")
                    nc.vector.tensor_mul(u[:, :], ps[:, :], sin_sb[:, q0:q0 + HALF])
                    c = rope_tmp.tile([128, HALF], F32, tag="c")
                    nc.vector.tensor_mul(c[:, :], ps[:, :], cos_sb[:, q0:q0 + HALF])
                    # rotation matmul overwrites the (now fully read) qkv
                    # psum bank instead of taking a fresh one
                    nc.tensor.matmul(ps[:, :], lhsT=p2t_sb[:, :], rhs=u[:, :],
                                     start=True, stop=True)
                    nc.vector.tensor_add(dest[:, pr, q0:q0 + HALF], c[:, :], ps[:, :])

                def qk_proj_step(ft, h2):
                    """q/k projection + rope for feature tile ft, one half."""
                    ps = qk_mains(ft, h2)
                    qk_crosses(ft, h2, ps)
                    qk_rope_finish(ft, h2, ps)

                def attention_steps(p, h2):
                    """List of emission closures forming a stateful pipeline:
                    each step takes the pending (j, c0, e) and emits its PV
                    after issuing the next QK+exp, so PE always has the next
                    QK queued while waiting on ACT."""
                    steps = []
                    if True:
                        q0 = h2 * HALF
                        js = list(active_j(h2))
                        pv = [pv_ps.tile([65, HALF], F32, tag=f"pv{hh}",
                                         name=f"pv{hh}_{p}_{h2}")
                              for hh in range(2)]

                        def emit_pv(j, c0, e, js=tuple(js), pv=pv):
                            for hh in range(2):
                                nc.tensor.matmul(
                                    pv[hh][0:65, c0:HALF],
                                    lhsT=v_sb[:, j, 2 * p + hh, :],
                                    rhs=e[:, hh, c0:HALF],
                                    start=(j == js[0]), stop=(j == js[-1]))

                        def qk_exp(j, h2=h2, q0=q0):
                            c0 = col0(j, h2)
                            qk = qk_ps.tile([128, 2, HALF], F32, tag="qk",
                                            name=f"qk_{p}_{h2}_{j}")
                            for hh in range(2):
                                base = hh * 64
                                nc.tensor.matmul(
                                    qk[:, hh, c0:HALF],
                                    lhsT=kT_sb[base:base + 64, p, j * 128:(j + 1) * 128],
                                    rhs=qT_sb[base:base + 64, p, q0 + c0:q0 + HALF],
                                    start=True, stop=True)
                            e = exp_sb.tile([128, 2, HALF], BF16, tag="e",
                                            name=f"e_{p}_{h2}_{j}")
                            if mode == "general":
                                t = mtmp.tile([128, 2, HALF], F32, tag="mtg")
                                for hh in range(2):
                                    nc.vector.scalar_tensor_tensor(
                                        t[:, hh, :], in0=qk[:, hh, :], scalar=SCALE,
                                        in1=mb_sb[:, j, q0:q0 + HALF],
                                        op0=mybir.AluOpType.mult,
                                        op1=mybir.AluOpType.add)
                                nc.scalar.activation(
                                    e[:, :, :], t[:, :, :],
                                    mybir.ActivationFunctionType.Exp)
                            else:
                                nc.scalar.activation(
                                    e[:, :, c0:HALF], qk[:, :, c0:HALF],
                                    mybir.ActivationFunctionType.Exp,
                                    scale=SCALE)
                                if causal and j * 128 >= q0:
                                    # zero masked lower-triangle of the diag
                                    # block (bf16 2x DVE), per head
                                    for hh in range(2):
                                        nc.vector.tensor_mul(
                                            e[:, hh, c0:c0 + 128],
                                            e[:, hh, c0:c0 + 128], m01_sb[:, :])
                            return (j, c0, e)

                        def make_step(j, qk_exp=qk_exp, emit_pv=emit_pv):
                            def step(state):
                                pend = qk_exp(j)
                                if state is not None:
                                    emit_pv(*state)
                                return pend
                            return step

                        for j in js:
                            steps.append(make_step(j))

                        def normalize(state, h2=h2, q0=q0, pv=pv,
                                      emit_pv=emit_pv):
                            if state is not None:
                                emit_pv(*state)
                            # one copy per head frees the PV psum banks;
                            # custom-DVE recip needs partition-0 SBUF f32
                            un = norm.tile([65, 2, HALF], F32, tag="un",
                                           name=f"un_{p}_{h2}")
                            nc.scalar.copy(un[0:65, 0, :], pv[0][0:65, :])
                            nc.vector.tensor_copy(un[0:65, 1, :], pv[1][0:65, :])
                            if debug:
                                nc.vector.tensor_copy(
                                    un_dump[0:65, p, :, q0:q0 + HALF],
                                    un[0:65, :, :])
                            db = norm.tile([1, 2, HALF], F32, tag="db")
                            nc.vector.tensor_copy(db[0:1, :, :], un[64:65, :, :])
                            r = norm.tile([1, 2, HALF], F32, tag="r")
                            nc.vector.reciprocal_approx_fast(
                                out=r[0:1, :, :], in_=db[0:1, :, :])
                            last = (p == NPAIR - 1 and h2 == 1)
                            for hh in range(2):
                                rb = norm.tile([64, HALF], F32, tag="rb")
                                nc.gpsimd.partition_broadcast(rb[:, :],
                                                              r[0:1, hh, :])
                                # final normalize gates out-proj: use the
                                # faster DVE for it
                                eng = nc.vector if last else nc.gpsimd
                                eng.tensor_mul(
                                    attnT_sb[hh * 64:(hh + 1) * 64, p,
                                             q0:q0 + HALF],
                                    un[0:64, hh, :], rb[:, :])
                            return None
                        steps.append(normalize)
                    return steps

                def run_interleaved(att_steps, fillers):
                    """Drive the attention pipeline with filler emission
                    (projections for later pairs) spread between steps, so
                    the in-order PE has independent matmuls behind any
                    ACT-gated PV."""
                    state = None
                    n, m = len(att_steps), len(fillers)
                    fi = 0
                    for i, step in enumerate(att_steps):
                        state = step(state)
                        target = min(m, ((i + 2) * m) // n)
                        while fi < target:
                            fillers[fi]()
                            fi += 1
                    assert state is None and fi == m

                def mk_v(st):
                    return lambda: v_proj_step(st)

                def mk_qk(ft, h2):
                    return lambda: qk_proj_step(ft, h2)

                y_dram = y.ap().rearrange("(t p) n -> p t n", p=128)

                def st_full(st, pool="qkv", final=False):
                    """Out-proj sequence tile inside the psum pool scope (a
                    pool-close barrier would stall the tail behind the whole
                    attention drain). One batched y DMA per tile; store
                    copies alternate ACT/DVE; the final tile DMAs per half."""
                    if pool == "qkv":
                        halves = [qkv_ps.tile([128, HALF], F32, tag="qkv",
                                              name=f"yh{st}_{n2}")[:, :]
                                  for n2 in range(2)]
                    else:
                        yp2 = qk_ps.tile([128, 2, HALF], F32, tag="qk",
                                         name=f"yt{st}")
                        halves = [yp2[:, 0, :], yp2[:, 1, :]]
                    for n2 in range(2):
                        sl = slice(n2 * HALF, (n2 + 1) * HALF)
                        yp = halves[n2]
                        for pq in range(NPAIR):
                            nc.tensor.matmul(
                                yp,
                                lhsT=attnT_sb[:, pq, st * 128:(st + 1) * 128],
                                rhs=woutT_sb[:, pq, sl],
                                start=(pq == 0), stop=(pq == NPAIR - 1))
                        if n2 == 0:
                            nc.scalar.copy(y_sb[:, st, sl], yp)
                        else:
                            nc.vector.tensor_copy(y_sb[:, st, sl], yp)
                        if final:
                            nc.sync.dma_start(out=y_dram[:, st, sl],
                                              in_=y_sb[:, st, sl])
                    if not final:
                        nc.sync.dma_start(out=y_dram[:, st, :],
                                          in_=y_sb[:, st, :])

                def mk_st(st):
                    return lambda: st_full(st)

                # pair-0 half-0 projections and the first v tiles up front.
                # NOTE: emission order is execution-order semantics under
                # Tile, so every v tile a PV step consumes must be emitted
                # before that step. Causal h2=0 only touches v[0:4]; the
                # other modes touch all 8 from the first group.
                if causal:
                    # short p-state warmup: burn the cold 0.65/1.2 GHz ramp
                    # on dummies while the first DMAs are in flight
                    for wi in range(7):
                        wp = qk_ps.tile([128, 2, HALF], F32, tag="qk",
                                        name=f"warm{wi}")
                        nc.tensor.matmul(wp[:, 0, :], lhsT=warm_sb[:, 0:128],
                                         rhs=warm_sb[:, :],
                                         start=True, stop=True)
                    # hi-plane mains first (their DMAs land first), then the
                    # lo-plane cross terms, then rope epilogues
                    psA = qk_mains(0, 0, skip=True)
                    psB = qk_mains(4, 0, skip=True)
                    qk_crosses(0, 0, psA, skip=True)
                    qk_crosses(4, 0, psB, skip=True)
                    qk_rope_finish(0, 0, psA)
                    qk_rope_finish(4, 0, psB)
                    # v after the ropes: wv lands behind the qk stream
                    v_proj_step(0)
                    v_proj_step(1)
                    fill_by_group = {
                        (0, 0): [mk_v(2), mk_qk(0, 1), mk_v(3), mk_qk(4, 1)],
                        (0, 1): [mk_qk(1, 0), mk_v(4), mk_qk(5, 0),
                                 mk_v(5), mk_qk(1, 1), mk_v(6),
                                 mk_qk(5, 1), mk_v(7)],
                        (1, 0): [mk_qk(2, 0), mk_qk(6, 0)],
                        (1, 1): [mk_qk(2, 1), mk_qk(6, 1)],
                        (2, 0): [mk_qk(3, 0), mk_qk(7, 0)],
                        (2, 1): [],
                        (3, 0): [mk_qk(3, 1), mk_qk(7, 1)],
                        (3, 1): [],
                    }
                else:
                    # non-causal: queries attend the full key range, so a
                    # pair's q/k projections (both halves) must precede its
                    # first attention group
                    qk_proj_step(0, 0)
                    qk_proj_step(4, 0)
                    qk_proj_step(0, 1)
                    qk_proj_step(4, 1)
                    for st in range(KT):
                        v_proj_step(st)
                    fill_by_group = {
                        (0, 0): [mk_qk(1, 0), mk_qk(5, 0)],
                        (0, 1): [mk_qk(1, 1), mk_qk(5, 1)],
                        (1, 0): [mk_qk(2, 0), mk_qk(6, 0)],
                        (1, 1): [mk_qk(2, 1), mk_qk(6, 1)],
                        (2, 0): [mk_qk(3, 0), mk_qk(7, 0)],
                        (2, 1): [mk_qk(3, 1), mk_qk(7, 1)],
                        (3, 0): [],
                        (3, 1): [],
                    }
                for p in range(NPAIR):
                    for h2 in range(2):
                        run_interleaved(attention_steps(p, h2),
                                        fill_by_group[(p, h2)])
                if causal:
                    # st 0..3 don't depend on the last group's normalize:
                    # their matmuls overlap the final recip/mul chain on DVE
                    for st in range(KT):
                        st_full(st, pool="qk" if st % 2 == 0 else "qkv",
                                final=(st == KT - 1))



            # ---------------- out projection ----------------
            if debug:
                nc.sync.dma_start(out=dbg["q"].ap(), in_=qT_sb[:, :, :])
                nc.sync.dma_start(out=dbg["k"].ap(), in_=kT_sb[:, :, :])
                nc.sync.dma_start(out=dbg["v"].ap(), in_=v_sb[:, :, :, :])
                nc.sync.dma_start(out=dbg["at"].ap(), in_=attnT_sb[:, :, :])
                nc.sync.dma_start(out=dbg["un"].ap(), in_=un_dump[:, :, :, :])

            y_dram = y.ap().rearrange("(t p) n -> p t n", p=128)
            st_tail = [] if causal else range(KT)
            with tc.tile_pool(name="y_ps", bufs=3, space="PSUM") as y_ps:
                for st in st_tail:
                    yp = y_ps.tile([128, D], F32, tag="yp", name=f"yp{st}")
                    for n2 in range(2):
                        sl = slice(n2 * HALF, (n2 + 1) * HALF)
                        for p in range(NPAIR):
                            nc.tensor.matmul(
                                yp[:, sl],
                                lhsT=attnT_sb[:, p, st * 128:(st + 1) * 128],
                                rhs=woutT_sb[:, p, sl],
                                start=(p == 0), stop=(p == NPAIR - 1))
                        # store each half right behind its matmuls; the very
                        # last store goes on DVE so ACT/DVE drain in parallel
                        if st == KT - 1 and n2 == 1:
                            nc.vector.tensor_copy(y_sb[:, st, sl], yp[:, sl])
                        else:
                            nc.scalar.copy(y_sb[:, st, sl], yp[:, sl])
                        nc.sync.dma_start(out=y_dram[:, st, sl],
                                          in_=y_sb[:, st, sl])

    nc.compile()
    return nc


@functools.lru_cache(maxsize=None)
def _program(mode):
    return _build_program(mode)


def _rope_tables(positions):
    pos = np.clip(np.asarray(positions).astype(np.int64), 0, MAX_POS - 1)
    inv_freq = 1.0 / (ROPE_BASE ** (np.arange(0, DH, 2, dtype=np.float32) / DH))
    t = (pos.astype(np.float32)[:, None] * inv_freq[None, :].astype(np.float32))
    cos_h = np.cos(t.astype(np.float32)).astype(np.float32)   # [S, 32]
    sin_h = np.sin(t.astype(np.float32)).astype(np.float32)
    # 1/PSCALE folded in: the fp8 hi/lo projections compute 256x-scaled
    # psums (x pre-scaled by 8, w by 32)
    cosT2 = np.ascontiguousarray(
        np.tile(np.concatenate([cos_h.T, cos_h.T], 0), (2, 1)) / 256.0
        ).astype(ml_dtypes.bfloat16)
    sinT2 = np.ascontiguousarray(
        np.tile(np.concatenate([sin_h.T, sin_h.T], 0), (2, 1)) / 256.0
        ).astype(ml_dtypes.bfloat16)
    return cosT2, sinT2


def kernel(hidden_states, positions, causal_mask, Wqkv, Wout):
    hidden = np.asarray(hidden_states, dtype=np.float32)
    assert hidden.shape == (B, S, D), hidden.shape
    mask = np.asarray(causal_mask).astype(bool)
    Wqkv_ = np.asarray(Wqkv, dtype=np.float32)
    Wout_ = np.asarray(Wout, dtype=np.float32)

    triu = np.triu(np.ones((S, S), dtype=bool), 1)
    if np.array_equal(mask, triu):
        mode = "causal"
    elif not mask.any():
        mode = "nomask"
    else:
        mode = "general"
    nc = _program(mode)

    cosT2, sinT2 = _rope_tables(positions)

    bf = ml_dtypes.bfloat16
    f8 = ml_dtypes.float8_e4m3

    def split8(a, w_order):
        """fp8 hi/lo stack along dim1: (hi, lo) for weights, (lo, hi) for x"""
        hi = a.astype(f8)
        lo = (a - hi.astype(np.float32)).astype(f8)
        pair = (hi, lo) if w_order else (lo, hi)
        return np.ascontiguousarray(np.stack(pair, axis=1))

    wqk, wv, wo = {}, {}, {}
    for g in (0, 1):
        heads = range(g * HLOC, (g + 1) * HLOC)
        # pair-interleaved: [q-pair0 | k-pair0 | q-pair1 | k-pair1 | ...]
        rows = []
        for pr in range(NPAIR):
            hA, hB = list(heads)[2 * pr], list(heads)[2 * pr + 1]
            rows.append(np.concatenate([hA * DH + _PERM, hB * DH + _PERM]))
            rows.append(np.concatenate([D + hA * DH + _PERM,
                                        D + hB * DH + _PERM]))
        wqk[g] = split8(
            Wqkv_[np.concatenate(rows), :].T * 32.0, True)
        wv[g] = split8(
            Wqkv_[2 * D + g * 512:2 * D + (g + 1) * 512, :].T * 32.0, True)
        wo[g] = np.ascontiguousarray(Wout_[:, g * 512:(g + 1) * 512].T).astype(bf)
    x8 = {b: split8(hidden[b].T * 8.0, False) for b in range(B)}

    maskb = None
    if mode == "general":
        maskb = np.where(mask.T, NEG, 0.0).astype(ml_dtypes.bfloat16)

    in_maps = []
    for c in range(NCORES):
        b, g = divmod(c, 2)
        m = {
            "xT8": x8[b],
            "wqkT8": wqk[g],
            "wvT8": wv[g],
            "woutT": wo[g],
            "cosT2": cosT2,
            "sinT2": sinT2,
        }
        if mode == "general":
            m["maskb"] = maskb
        in_maps.append(m)

    global LAST_MODE
    LAST_MODE = mode
    # retry once on non-finite output: transient device-state glitches
    # (wedged-core class) can surface as NaN on an otherwise-correct run
    for attempt in range(2):
        res = run_bass_kernel_spmd(nc, in_maps, core_ids=list(range(NCORES)))
        y = np.empty((B, S, D), dtype=np.float32)
        for b in range(B):
            y[b] = (res.results[2 * b]["y"].astype(np.float32)
                    + res.results[2 * b + 1]["y"].astype(np.float32))
        if np.isfinite(y).all():
            break
    return y



# revision 56
# speedup vs baseline: 1.0276x; 1.0276x over previous
"""Multi-head self-attention (RoPE + causal SDPA) Trainium2 Bass kernel.

Sharding: 8 cores = 4 batches x 2 head-groups (8 heads each).
Per core: qkv projection (tensor-parallel columns), RoPE, attention for its
8 heads, out_proj against its 512 input columns -> partial y [S, D].
Host sums the two partials per batch (pairwise reduce) and stacks batches.

QKV projections run in fp8-e4m3 DoubleRow perf mode (2 K-tiles per
instruction, 0.5 cycles/row) with a host-side hi/lo residual split:
x = 8x_hi + 8x_lo, W = 32W_hi + 32W_lo (fp8 each), computing
hi@hi (paired K-tiles) + the (hi,lo)/(lo,hi) cross terms and dropping
lo@lo. This is bf16-accuracy at ~2/3 the matmul cycles; the 256x psum
scale is folded into the rope tables and the v-store copy scale.

Layouts on device (per core):
  xT    [D, S]   bf16  hidden[b].T
  wqkT  [D, 1024] bf16 pair-interleaved cols [q-pair0 |k-pair0 |q-pair1 |...];
                      within a 128-col block: head-even feats (rope-permuted)
                      0:64, head-odd 64:128
  wvT   [D, 512] bf16  v weights (unpermuted), local-head-major
  woutT [512, D] bf16  rows match attnT feature order
  cosT2/sinT2 [128, S] bf16  rope tables in permuted layout, tiled x2 for pairs
  y     [S, D]   bf16  partial output (host upconverts and adds pairs in f32)

RoPE permuted layout: per-head feats reordered [0,2,..62, 1,3,..63] so
rotate_half becomes a 32-partition block swap, implemented as a 128x128
block-diagonal matmul (p2t) on (q * sinT2).

Attention is computed transposed: scoresT[j] = kT_j^T @ qT -> [Sk=128, Sq<=512]
so softmax's reduction axis is the matmul contraction axis; exp via ACT;
denominator via an appended ones-column in the PV lhsT ([v | 1] -> psum row 64);
PV gives attnT [64, Sq] feature-major which feeds out_proj directly.
"""

import functools
import os
import sys

import numpy as np
import ml_dtypes

try:
    import concourse.bacc as bacc
except ImportError:
    for _p in ("/opt/trn_rl_repo", "/root/.axon_site/_ro/trn_rl_repo"):
        if os.path.isdir(_p) and _p not in sys.path:
            sys.path.append(_p)
    import concourse.bacc as bacc
import concourse.mybir as mybir
import concourse.tile as tile
from concourse.bass_utils import run_bass_kernel_spmd

BF16 = mybir.dt.bfloat16
F32 = mybir.dt.float32
F8 = mybir.dt.float8e4
DR = mybir.MatmulPerfMode.DoubleRow
PSCALE = 256.0  # host scales x by 8 and w by 32; rope tables and v-copy unscale

B, S, D = 4, 1024, 1024
H, DH = 16, 64
HLOC, NPAIR = 8, 4          # heads / head-pairs per core
KT = 8                      # Sk tiles of 128
DT = 8                      # contraction (D) tiles of 128
HALF = 512                  # Sq half width (psum bank = 512 f32)
SCALE = DH ** -0.5
NEG = -1e30
MAX_POS = 4096
ROPE_BASE = 10000.0
NCORES = 8

LAST_EXEC_NS = None
LAST_TRACE = None
LAST_MODE = None

_PERM = np.concatenate([np.arange(0, DH, 2), np.arange(1, DH, 2)])  # even|odd


def _p2t_np():
    """(block-diag rotation P, transposed) P @ q' = [-q'[32:64]; q'[0:32]] per 64-block."""
    p2 = np.zeros((128, 128), np.float32)
    for b in (0, 64):
        for i in range(32):
            p2[b + i, b + 32 + i] = -1.0
            p2[b + 32 + i, b + i] = 1.0
    return np.ascontiguousarray(p2.T).astype(ml_dtypes.bfloat16)


def _mask01_np():
    """Multiplicative diag mask (bf16): masked (k>q) -> 0, allowed -> 1."""
    r = np.arange(128)
    return np.where(r[:, None] > r[None, :], 0.0, 1.0).astype(ml_dtypes.bfloat16)


def _build_program(mode):
    """mode: 'causal' | 'nomask' | 'general'"""
    nc = bacc.Bacc(None, target_bir_lowering=False)
    # fp8 hi/lo pairs: x dim1 = (lo, hi), w dim1 = (hi, lo) so DoubleRow
    # cross products pair (Whi,xlo)+(Wlo,xhi) in one instruction
    xT8 = nc.dram_tensor("xT8", [D, 2, S], F8, kind="ExternalInput")
    wqkT8 = nc.dram_tensor("wqkT8", [D, 2, 1024], F8, kind="ExternalInput")
    wvT8 = nc.dram_tensor("wvT8", [D, 2, 512], F8, kind="ExternalInput")
    woutT = nc.dram_tensor("woutT", [512, D], BF16, kind="ExternalInput")
    cosT2 = nc.dram_tensor("cosT2", [128, S], BF16, kind="ExternalInput")
    sinT2 = nc.dram_tensor("sinT2", [128, S], BF16, kind="ExternalInput")
    # y partials in bf16: halves the output DMA bytes (the drain tail is
    # DMA-serialized); the host upconverts and sums pairs in f32
    y = nc.dram_tensor("y", [S, D], BF16, kind="ExternalOutput")
    maskb = None
    if mode == "general":
        maskb = nc.dram_tensor("maskb", [S, S], BF16, kind="ExternalInput")
    debug = bool(os.environ.get("KERNEL_DEBUG"))
    dbg = {}
    if debug:
        dbg["q"] = nc.dram_tensor("dbg_q", [128, NPAIR, S], BF16, kind="ExternalOutput")
        dbg["k"] = nc.dram_tensor("dbg_k", [128, NPAIR, S], BF16, kind="ExternalOutput")
        dbg["v"] = nc.dram_tensor("dbg_v", [128, KT, HLOC, 65], BF16, kind="ExternalOutput")
        dbg["at"] = nc.dram_tensor("dbg_at", [128, NPAIR, S], BF16, kind="ExternalOutput")
        dbg["un"] = nc.dram_tensor("dbg_un", [128, NPAIR, 2, S], F32, kind="ExternalOutput")
    p2t = nc.inline_tensor(_p2t_np(), name="p2t")
    m01 = nc.inline_tensor(_mask01_np(), name="mask01")

    causal = mode == "causal"

    def active_j(h2):
        return range(4 * (h2 + 1)) if causal else range(KT)

    def col0(j, h2):
        return max(0, j * 128 - h2 * HALF) if causal else 0

    with tile.TileContext(nc) as tc:
        with tc.tile_pool(name="persist", bufs=1) as pp:
            xT_sb = pp.tile([128, 2, DT, S], F8, tag="xT")
            wqkT_sb = pp.tile([128, 2, DT, 1024], F8, tag="wqkT")
            wvT_sb = pp.tile([128, 2, DT, 512], F8, tag="wvT")
            woutT_sb = pp.tile([128, NPAIR, D], BF16, tag="woutT")
            cos_sb = pp.tile([128, S], BF16, tag="cos")
            sin_sb = pp.tile([128, S], BF16, tag="sin")
            p2t_sb = pp.tile([128, 128], BF16, tag="p2t")
            m01_sb = pp.tile([128, 128], BF16, tag="m01")
            qT_sb = pp.tile([128, NPAIR, S], BF16, tag="qT")
            kT_sb = pp.tile([128, NPAIR, S], BF16, tag="kT")
            v_sb = pp.tile([128, KT, HLOC, 65], BF16, tag="v")
            attnT_sb = pp.tile([128, NPAIR, S], BF16, tag="attnT")
            y_sb = pp.tile([128, KT, D], BF16, tag="y")
            mb_sb = None
            un_dump = None
            if debug:
                un_dump = pp.tile([128, NPAIR, 2, S], F32, tag="und")

            # input DMAs, batched + consumption-ordered: each DMA costs
            # ~625ns of serial HWDGE time + ~2.2us fixed latency, so batch
            # into ~14 transfers ordered by first use
            xT_d = xT8.ap().rearrange("(t p) l s -> p l t s", p=128)
            wqkT_d = wqkT8.ap().rearrange("(t p) l n -> p l t n", p=128)
            wvT_d = wvT8.ap().rearrange("(t p) l n -> p l t n", p=128)
            woutT_d = woutT.ap().rearrange("(p r) n -> r p n", r=128)
            # DMA APs are limited to 3 dims: one transfer per hi/lo plane,
            # hi planes first (the DoubleRow main products read only hi)
            nc.sync.dma_start(out=wqkT_sb[:, 0, :, 0:512],
                              in_=wqkT_d[:, 0, :, 0:512])
            nc.sync.dma_start(out=xT_sb[:, 1, 0:4, 0:HALF],
                              in_=xT_d[:, 1, 0:4, 0:HALF])
            nc.sync.dma_start(out=xT_sb[:, 1, 4:8, 0:HALF],
                              in_=xT_d[:, 1, 4:8, 0:HALF])
            nc.sync.dma_start(out=wvT_sb[:, 0, :, :], in_=wvT_d[:, 0, :, :])
            nc.sync.dma_start(out=wqkT_sb[:, 1, 0:4, 0:512],
                              in_=wqkT_d[:, 1, 0:4, 0:512])
            nc.sync.dma_start(out=xT_sb[:, 0, 0:4, 0:HALF],
                              in_=xT_d[:, 0, 0:4, 0:HALF])
            nc.sync.dma_start(out=wqkT_sb[:, 1, 4:8, 0:512],
                              in_=wqkT_d[:, 1, 4:8, 0:512])
            nc.sync.dma_start(out=xT_sb[:, 0, 4:8, 0:HALF],
                              in_=xT_d[:, 0, 4:8, 0:HALF])
            nc.sync.dma_start(out=p2t_sb[:, :], in_=p2t.ap())
            nc.sync.dma_start(out=cos_sb[:, 0:HALF], in_=cosT2.ap()[:, 0:HALF])
            nc.sync.dma_start(out=sin_sb[:, 0:HALF], in_=sinT2.ap()[:, 0:HALF])
            nc.sync.dma_start(out=m01_sb[:, :], in_=m01.ap())
            nc.sync.dma_start(out=wvT_sb[:, 1, :, :], in_=wvT_d[:, 1, :, :])
            nc.sync.dma_start(out=xT_sb[:, 1, :, HALF:S],
                              in_=xT_d[:, 1, :, HALF:S])
            nc.sync.dma_start(out=xT_sb[:, 0, :, HALF:S],
                              in_=xT_d[:, 0, :, HALF:S])
            nc.sync.dma_start(out=cos_sb[:, HALF:S], in_=cosT2.ap()[:, HALF:S])
            nc.sync.dma_start(out=sin_sb[:, HALF:S], in_=sinT2.ap()[:, HALF:S])
            nc.sync.dma_start(out=wqkT_sb[:, 0, :, 512:1024],
                              in_=wqkT_d[:, 0, :, 512:1024])
            nc.sync.dma_start(out=wqkT_sb[:, 1, :, 512:1024],
                              in_=wqkT_d[:, 1, :, 512:1024])
            if mode == "general":
                mb_sb = pp.tile([128, KT, S], BF16, tag="mb")
                nc.sync.dma_start(
                    out=mb_sb[:, :, :],
                    in_=maskb.ap().rearrange("(t p) s -> p t s", p=128),
                )
            nc.sync.dma_start(out=woutT_sb[:, :, :], in_=woutT_d)

            # ones column for the PV denominator rows
            nc.vector.memset(v_sb[:, :, :, 64:65], 1.0)
            warm_sb = None
            if causal:
                # p-state warmup fodder: PE ramps 0.65 -> 1.2 -> 2.4 GHz over
                # ~3us of continuous execution; burn the DMA-latency window
                # on dummy matmuls so real work runs at full speed
                warm_sb = pp.tile([128, 512], BF16, tag="warm")
                nc.vector.memset(warm_sb[:, :], 0.0)

            with tc.tile_pool(name="qkv_ps", bufs=2, space="PSUM") as qkv_ps, \
                 tc.tile_pool(name="qk_ps", bufs=2, space="PSUM") as qk_ps, \
                 tc.tile_pool(name="pv_ps", bufs=1, space="PSUM") as pv_ps, \
                 tc.tile_pool(name="rope_tmp", bufs=2 if mode == "general" else 4) as rope_tmp, \
                 tc.tile_pool(name="exp_sb", bufs=3 if mode == "general" else 8) as exp_sb, \
                 tc.tile_pool(name="mtmp", bufs=2) as mtmp, \
                 tc.tile_pool(name="norm", bufs=2 if mode == "general" else 4) as norm:

                def v_mains(st, skip=False, pool=None, ps=None):
                    # alternate between the qkv slots and the (still idle)
                    # attention qk slots so four accumulators rotate
                    if ps is not None:
                        pass
                    elif pool == "qk":
                        ps2 = qk_ps.tile([128, 2, HALF], F32, tag="qk",
                                         name=f"vp{st}")
                        ps = ps2[:, 0, :]
                    elif st % 2 == 0:
                        ps = qkv_ps.tile([128, HALF], F32, tag="qkv",
                                         name=f"vp{st}")
                    else:
                        ps2 = qk_ps.tile([128, 2, HALF], F32, tag="qk",
                                         name=f"vp{st}")
                        ps = ps2[:, 0, :]
                    sl = slice(st * 128, (st + 1) * 128)
                    for dp in range(DT // 2):
                        nc.tensor.matmul(
                            ps[:, :],
                            lhsT=xT_sb[:, 1, 2 * dp:2 * dp + 2, sl],
                            rhs=wvT_sb[:, 0, 2 * dp:2 * dp + 2, :],
                            start=(dp == 0), stop=False, perf_mode=DR,
                            skip_group_check=skip)
                    return ps

                def v_crosses(st, ps, skip=False):
                    sl = slice(st * 128, (st + 1) * 128)
                    for d in range(DT):
                        nc.tensor.matmul(
                            ps[:, :],
                            lhsT=xT_sb[:, :, d, sl],
                            rhs=wvT_sb[:, :, d, :],
                            start=False, stop=(d == DT - 1), perf_mode=DR,
                            skip_group_check=skip)
                    nc.scalar.activation(
                        v_sb[:, st, :, 0:64],
                        ps.rearrange("p (h f) -> p h f", h=HLOC),
                        mybir.ActivationFunctionType.Copy, scale=1.0 / PSCALE)

                def v_proj_step(st):
                    v_crosses(st, v_mains(st))

                def qk_mains(ft, h2, skip=False, pool="qkv"):
                    pr = ft % 4
                    # wqkT is pair-interleaved: [q-pair0 |k-pair0 |q-pair1 ...]
                    qk_col = pr * 256 + (0 if ft < 4 else 128)
                    q0 = h2 * HALF
                    if pool == "qk":
                        ps = qk_ps.tile([128, 2, HALF], F32, tag="qk",
                                        name=f"qkp{ft}_{h2}")[:, 0, :]
                    else:
                        ps = qkv_ps.tile([128, HALF], F32, tag="qkv",
                                         name=f"qkp{ft}_{h2}")
                    qsl = slice(qk_col, qk_col + 128)
                    for dp in range(DT // 2):
                        nc.tensor.matmul(
                            ps[:, :],
                            lhsT=wqkT_sb[:, 0, 2 * dp:2 * dp + 2, qsl],
                            rhs=xT_sb[:, 1, 2 * dp:2 * dp + 2, q0:q0 + HALF],
                            start=(dp == 0), stop=False, perf_mode=DR,
                            skip_group_check=skip)
                    return ps

                def qk_crosses(ft, h2, ps, skip=False):
                    pr = ft % 4
                    qk_col = pr * 256 + (0 if ft < 4 else 128)
                    q0 = h2 * HALF
                    qsl = slice(qk_col, qk_col + 128)
                    for d in range(DT):
                        nc.tensor.matmul(
                            ps[:, :],
                            lhsT=wqkT_sb[:, :, d, qsl],
                            rhs=xT_sb[:, :, d, q0:q0 + HALF],
                            start=False, stop=(d == DT - 1), perf_mode=DR,
                            skip_group_check=skip)

                def qk_rope_finish(ft, h2, ps):
                    dest = qT_sb if ft < 4 else kT_sb
                    pr = ft % 4
                    q0 = h2 * HALF
                    u = rope_tmp.tile([128, HALF], BF16, tag="u")
                    nc.vector.tensor_mul(u[:, :], ps[:, :], sin_sb[:, q0:q0 + HALF])
                    c = rope_tmp.tile([128, HALF], F32, tag="c")
                    nc.vector.tensor_mul(c[:, :], ps[:, :], cos_sb[:, q0:q0 + HALF])
                    # rotation matmul overwrites the (now fully read) qkv
                    # psum bank instead of taking a fresh one
                    nc.tensor.matmul(ps[:, :], lhsT=p2t_sb[:, :], rhs=u[:, :],
                                     start=True, stop=True)
                    nc.vector.tensor_add(dest[:, pr, q0:q0 + HALF], c[:, :], ps[:, :])

                def qk_proj_step(ft, h2):
                    """q/k projection + rope for feature tile ft, one half."""
                    ps = qk_mains(ft, h2)
                    qk_crosses(ft, h2, ps)
                    qk_rope_finish(ft, h2, ps)

                def attention_steps(p, h2):
                    """List of emission closures forming a stateful pipeline:
                    each step takes the pending (j, c0, e) and emits its PV
                    after issuing the next QK+exp, so PE always has the next
                    QK queued while waiting on ACT."""
                    steps = []
                    if True:
                        q0 = h2 * HALF
                        js = list(active_j(h2))
                        pv = [pv_ps.tile([128, HALF], F32, tag=f"pv{hh}",
                                         name=f"pv{hh}_{p}_{h2}")
                              for hh in range(2)]

                        def emit_pv(j, c0, e, js=tuple(js), pv=pv):
                            for hh in range(2):
                                nc.tensor.matmul(
                                    pv[hh][0:65, c0:HALF],
                                    lhsT=v_sb[:, j, 2 * p + hh, :],
                                    rhs=e[:, hh, c0:HALF],
                                    start=(j == js[0]), stop=(j == js[-1]))

                        def qk_exp(j, h2=h2, q0=q0):
                            c0 = col0(j, h2)
                            qk = qk_ps.tile([128, 2, HALF], F32, tag="qk",
                                            name=f"qk_{p}_{h2}_{j}")
                            for hh in range(2):
                                base = hh * 64
                                nc.tensor.matmul(
                                    qk[:, hh, c0:HALF],
                                    lhsT=kT_sb[base:base + 64, p, j * 128:(j + 1) * 128],
                                    rhs=qT_sb[base:base + 64, p, q0 + c0:q0 + HALF],
                                    start=True, stop=True)
                            e = exp_sb.tile([128, 2, HALF], BF16, tag="e",
                                            name=f"e_{p}_{h2}_{j}")
                            if mode == "general":
                                t = mtmp.tile([128, 2, HALF], F32, tag="mtg")
                                for hh in range(2):
                                    nc.vector.scalar_tensor_tensor(
                                        t[:, hh, :], in0=qk[:, hh, :], scalar=SCALE,
                                        in1=mb_sb[:, j, q0:q0 + HALF],
                                        op0=mybir.AluOpType.mult,
                                        op1=mybir.AluOpType.add)
                                nc.scalar.activation(
                                    e[:, :, :], t[:, :, :],
                                    mybir.ActivationFunctionType.Exp)
                            else:
                                nc.scalar.activation(
                                    e[:, :, c0:HALF], qk[:, :, c0:HALF],
                                    mybir.ActivationFunctionType.Exp,
                                    scale=SCALE)
                                if causal and j * 128 >= q0:
                                    # zero masked lower-triangle of the diag
                                    # block (bf16 2x DVE), per head
                                    for hh in range(2):
                                        nc.vector.tensor_mul(
                                            e[:, hh, c0:c0 + 128],
                                            e[:, hh, c0:c0 + 128], m01_sb[:, :])
                            return (j, c0, e)

                        def make_step(j, qk_exp=qk_exp, emit_pv=emit_pv):
                            def step(state):
                                pend = qk_exp(j)
                                if state is not None:
                                    emit_pv(*state)
                                return pend
                            return step

                        for j in js:
                            steps.append(make_step(j))

                        def normalize(state, h2=h2, q0=q0, pv=pv,
                                      emit_pv=emit_pv):
                            if state is not None:
                                emit_pv(*state)
                            # one copy per head frees the PV psum banks;
                            # custom-DVE recip needs partition-0 SBUF f32
                            un = norm.tile([65, 2, HALF], F32, tag="un",
                                           name=f"un_{p}_{h2}")
                            nc.scalar.copy(un[0:65, 0, :], pv[0][0:65, :])
                            nc.vector.tensor_copy(un[0:65, 1, :], pv[1][0:65, :])
                            if debug:
                                nc.vector.tensor_copy(
                                    un_dump[0:65, p, :, q0:q0 + HALF],
                                    un[0:65, :, :])
                            db = norm.tile([1, 2, HALF], F32, tag="db")
                            nc.vector.tensor_copy(db[0:1, :, :], un[64:65, :, :])
                            r = norm.tile([1, 2, HALF], F32, tag="r")
                            nc.vector.reciprocal_approx_fast(
                                out=r[0:1, :, :], in_=db[0:1, :, :])
                            last = (p == NPAIR - 1 and h2 == 1)
                            for hh in range(2):
                                rb = norm.tile([64, HALF], F32, tag="rb")
                                nc.gpsimd.partition_broadcast(rb[:, :],
                                                              r[0:1, hh, :])
                                # final normalize gates out-proj: use the
                                # faster DVE for it
                                eng = nc.vector if last else nc.gpsimd
                                eng.tensor_mul(
                                    attnT_sb[hh * 64:(hh + 1) * 64, p,
                                             q0:q0 + HALF],
                                    un[0:64, hh, :], rb[:, :])
                            return None
                        steps.append(normalize)
                    return steps

                def run_interleaved(att_steps, fillers):
                    """Drive the attention pipeline with filler emission
                    (projections for later pairs) spread between steps, so
                    the in-order PE has independent matmuls behind any
                    ACT-gated PV."""
                    state = None
                    n, m = len(att_steps), len(fillers)
                    fi = 0
                    for i, step in enumerate(att_steps):
                        state = step(state)
                        target = min(m, ((i + 2) * m) // n)
                        while fi < target:
                            fillers[fi]()
                            fi += 1
                    assert state is None and fi == m

                def mk_v(st):
                    return lambda: v_proj_step(st)

                def mk_qk(ft, h2):
                    return lambda: qk_proj_step(ft, h2)

                y_dram = y.ap().rearrange("(t p) n -> p t n", p=128)

                def st_full(st, pool="qkv", final=False):
                    """Out-proj sequence tile inside the psum pool scope (a
                    pool-close barrier would stall the tail behind the whole
                    attention drain). One batched y DMA per tile; store
                    copies alternate ACT/DVE; the final tile DMAs per half."""
                    if pool == "qkv":
                        halves = [qkv_ps.tile([128, HALF], F32, tag="qkv",
                                              name=f"yh{st}_{n2}")[:, :]
                                  for n2 in range(2)]
                    else:
                        yp2 = qk_ps.tile([128, 2, HALF], F32, tag="qk",
                                         name=f"yt{st}")
                        halves = [yp2[:, 0, :], yp2[:, 1, :]]
                    for n2 in range(2):
                        sl = slice(n2 * HALF, (n2 + 1) * HALF)
                        yp = halves[n2]
                        for pq in range(NPAIR):
                            nc.tensor.matmul(
                                yp,
                                lhsT=attnT_sb[:, pq, st * 128:(st + 1) * 128],
                                rhs=woutT_sb[:, pq, sl],
                                start=(pq == 0), stop=(pq == NPAIR - 1))
                        if n2 == 0:
                            nc.scalar.copy(y_sb[:, st, sl], yp)
                        else:
                            nc.vector.tensor_copy(y_sb[:, st, sl], yp)
                        if final:
                            nc.sync.dma_start(out=y_dram[:, st, sl],
                                              in_=y_sb[:, st, sl])
                    if not final:
                        nc.sync.dma_start(out=y_dram[:, st, :],
                                          in_=y_sb[:, st, :])

                def mk_st(st):
                    return lambda: st_full(st)

                # pair-0 half-0 projections and the first v tiles up front.
                # NOTE: emission order is execution-order semantics under
                # Tile, so every v tile a PV step consumes must be emitted
                # before that step. Causal h2=0 only touches v[0:4]; the
                # other modes touch all 8 from the first group.
                if causal:
                    # short p-state warmup: burn the cold 0.65/1.2 GHz ramp
                    # on dummies while the first DMAs are in flight
                    for wi in range(7):
                        wp = qk_ps.tile([128, 2, HALF], F32, tag="qk",
                                        name=f"warm{wi}")
                        nc.tensor.matmul(wp[:, 0, :], lhsT=warm_sb[:, 0:128],
                                         rhs=warm_sb[:, :],
                                         start=True, stop=True)
                    # hi-plane mains first (their DMAs land first), then the
                    # lo-plane cross terms, then rope epilogues
                    # six hi-plane accumulations open at once (qkv 2 +
                    # qk 4 + pv 2 banks): PE chews mains while the lo planes
                    # stream in, then crosses, ropes, v epilogues
                    psA = qk_mains(0, 0, skip=True)
                    psB = qk_mains(4, 0, skip=True)
                    psC = qk_mains(1, 0, skip=True, pool="qk")
                    psD = qk_mains(5, 0, skip=True, pool="qk")
                    vh0 = pv_ps.tile([128, HALF], F32, tag="pv0",
                                     name="vh0")
                    vh1 = pv_ps.tile([128, HALF], F32, tag="pv1",
                                     name="vh1")
                    v_mains(0, skip=True, ps=vh0[:, :])
                    v_mains(1, skip=True, ps=vh1[:, :])
                    qk_crosses(0, 0, psA, skip=True)
                    qk_crosses(4, 0, psB, skip=True)
                    qk_crosses(1, 0, psC, skip=True)
                    qk_crosses(5, 0, psD, skip=True)
                    qk_rope_finish(0, 0, psA)
                    qk_rope_finish(4, 0, psB)
                    qk_rope_finish(1, 0, psC)
                    qk_rope_finish(5, 0, psD)
                    v_crosses(0, vh0[:, :], skip=True)
                    v_crosses(1, vh1[:, :], skip=True)
                    fill_by_group = {
                        (0, 0): [mk_v(2), mk_qk(0, 1), mk_v(3), mk_qk(4, 1)],
                        (0, 1): [mk_v(4), mk_qk(1, 1), mk_v(5),
                                 mk_v(6), mk_qk(5, 1), mk_v(7)],
                        (1, 0): [mk_qk(2, 0), mk_qk(6, 0)],
                        (1, 1): [mk_qk(2, 1), mk_qk(6, 1)],
                        (2, 0): [mk_qk(3, 0), mk_qk(7, 0)],
                        (2, 1): [],
                        (3, 0): [mk_qk(3, 1), mk_qk(7, 1)],
                        (3, 1): [],
                    }
                else:
                    # non-causal: queries attend the full key range, so a
                    # pair's q/k projections (both halves) must precede its
                    # first attention group
                    qk_proj_step(0, 0)
                    qk_proj_step(4, 0)
                    qk_proj_step(0, 1)
                    qk_proj_step(4, 1)
                    for st in range(KT):
                        v_proj_step(st)
                    fill_by_group = {
                        (0, 0): [mk_qk(1, 0), mk_qk(5, 0)],
                        (0, 1): [mk_qk(1, 1), mk_qk(5, 1)],
                        (1, 0): [mk_qk(2, 0), mk_qk(6, 0)],
                        (1, 1): [mk_qk(2, 1), mk_qk(6, 1)],
                        (2, 0): [mk_qk(3, 0), mk_qk(7, 0)],
                        (2, 1): [mk_qk(3, 1), mk_qk(7, 1)],
                        (3, 0): [],
                        (3, 1): [],
                    }
                for p in range(NPAIR):
                    for h2 in range(2):
                        run_interleaved(attention_steps(p, h2),
                                        fill_by_group[(p, h2)])
                if causal:
                    # st 0..3 don't depend on the last group's normalize:
                    # their matmuls overlap the final recip/mul chain on DVE
                    for st in range(KT):
                        st_full(st, pool="qk" if st % 2 == 0 else "qkv",
                                final=(st == KT - 1))



            # ---------------- out projection ----------------
            if debug:
                nc.sync.dma_start(out=dbg["q"].ap(), in_=qT_sb[:, :, :])
                nc.sync.dma_start(out=dbg["k"].ap(), in_=kT_sb[:, :, :])
                nc.sync.dma_start(out=dbg["v"].ap(), in_=v_sb[:, :, :, :])
                nc.sync.dma_start(out=dbg["at"].ap(), in_=attnT_sb[:, :, :])
                nc.sync.dma_start(out=dbg["un"].ap(), in_=un_dump[:, :, :, :])

            y_dram = y.ap().rearrange("(t p) n -> p t n", p=128)
            st_tail = [] if causal else range(KT)
            with tc.tile_pool(name="y_ps", bufs=3, space="PSUM") as y_ps:
                for st in st_tail:
                    yp = y_ps.tile([128, D], F32, tag="yp", name=f"yp{st}")
                    for n2 in range(2):
                        sl = slice(n2 * HALF, (n2 + 1) * HALF)
                        for p in range(NPAIR):
                            nc.tensor.matmul(
                                yp[:, sl],
                                lhsT=attnT_sb[:, p, st * 128:(st + 1) * 128],
                                rhs=woutT_sb[:, p, sl],
                                start=(p == 0), stop=(p == NPAIR - 1))
                        # store each half right behind its matmuls; the very
                        # last store goes on DVE so ACT/DVE drain in parallel
                        if st == KT - 1 and n2 == 1:
                            nc.vector.tensor_copy(y_sb[:, st, sl], yp[:, sl])
                        else:
                            nc.scalar.copy(y_sb[:, st, sl], yp[:, sl])
                        nc.sync.dma_start(out=y_dram[:, st, sl],
                                          in_=y_sb[:, st, sl])

    nc.compile()
    return nc


@functools.lru_cache(maxsize=None)
def _program(mode):
    return _build_program(mode)


def _rope_tables(positions):
    pos = np.clip(np.asarray(positions).astype(np.int64), 0, MAX_POS - 1)
    inv_freq = 1.0 / (ROPE_BASE ** (np.arange(0, DH, 2, dtype=np.float32) / DH))
    t = (pos.astype(np.float32)[:, None] * inv_freq[None, :].astype(np.float32))
    cos_h = np.cos(t.astype(np.float32)).astype(np.float32)   # [S, 32]
    sin_h = np.sin(t.astype(np.float32)).astype(np.float32)
    # 1/PSCALE folded in: the fp8 hi/lo projections compute 256x-scaled
    # psums (x pre-scaled by 8, w by 32)
    cosT2 = np.ascontiguousarray(
        np.tile(np.concatenate([cos_h.T, cos_h.T], 0), (2, 1)) / 256.0
        ).astype(ml_dtypes.bfloat16)
    sinT2 = np.ascontiguousarray(
        np.tile(np.concatenate([sin_h.T, sin_h.T], 0), (2, 1)) / 256.0
        ).astype(ml_dtypes.bfloat16)
    return cosT2, sinT2


def kernel(hidden_states, positions, causal_mask, Wqkv, Wout):
    hidden = np.asarray(hidden_states, dtype=np.float32)
    assert hidden.shape == (B, S, D), hidden.shape
    mask = np.asarray(causal_mask).astype(bool)
    Wqkv_ = np.asarray(Wqkv, dtype=np.float32)
    Wout_ = np.asarray(Wout, dtype=np.float32)

    triu = np.triu(np.ones((S, S), dtype=bool), 1)
    if np.array_equal(mask, triu):
        mode = "causal"
    elif not mask.any():
        mode = "nomask"
    else:
        mode = "general"
    nc = _program(mode)

    cosT2, sinT2 = _rope_tables(positions)

    bf = ml_dtypes.bfloat16
    f8 = ml_dtypes.float8_e4m3

    def split8(a, w_order):
        """fp8 hi/lo stack along dim1: (hi, lo) for weights, (lo, hi) for x"""
        hi = a.astype(f8)
        lo = (a - hi.astype(np.float32)).astype(f8)
        pair = (hi, lo) if w_order else (lo, hi)
        return np.ascontiguousarray(np.stack(pair, axis=1))

    wqk, wv, wo = {}, {}, {}
    for g in (0, 1):
        heads = range(g * HLOC, (g + 1) * HLOC)
        # pair-interleaved: [q-pair0 | k-pair0 | q-pair1 | k-pair1 | ...]
        rows = []
        for pr in range(NPAIR):
            hA, hB = list(heads)[2 * pr], list(heads)[2 * pr + 1]
            rows.append(np.concatenate([hA * DH + _PERM, hB * DH + _PERM]))
            rows.append(np.concatenate([D + hA * DH + _PERM,
                                        D + hB * DH + _PERM]))
        wqk[g] = split8(
            Wqkv_[np.concatenate(rows), :].T * 32.0, True)
        wv[g] = split8(
            Wqkv_[2 * D + g * 512:2 * D + (g + 1) * 512, :].T * 32.0, True)
        wo[g] = np.ascontiguousarray(Wout_[:, g * 512:(g + 1) * 512].T).astype(bf)
    x8 = {b: split8(hidden[b].T * 8.0, False) for b in range(B)}

    maskb = None
    if mode == "general":
        maskb = np.where(mask.T, NEG, 0.0).astype(ml_dtypes.bfloat16)

    in_maps = []
    for c in range(NCORES):
        b, g = divmod(c, 2)
        m = {
            "xT8": x8[b],
            "wqkT8": wqk[g],
            "wvT8": wv[g],
            "woutT": wo[g],
            "cosT2": cosT2,
            "sinT2": sinT2,
        }
        if mode == "general":
            m["maskb"] = maskb
        in_maps.append(m)

    global LAST_MODE
    LAST_MODE = mode
    # retry once on non-finite output: transient device-state glitches
    # (wedged-core class) can surface as NaN on an otherwise-correct run
    for attempt in range(2):
        res = run_bass_kernel_spmd(nc, in_maps, core_ids=list(range(NCORES)))
        y = np.empty((B, S, D), dtype=np.float32)
        for b in range(B):
            y[b] = (res.results[2 * b]["y"].astype(np.float32)
                    + res.results[2 * b + 1]["y"].astype(np.float32))
        if np.isfinite(y).all():
            break
    return y

